# revision 48
# baseline (speedup 1.0000x reference)
"""Trainium2 Bass kernel for GQA attention prefill (Mistral-style, RoPE, causal).

B=1, S=2048, DIM=4096, 32 Q heads / 8 KV heads, HD=128, rope theta 1e6.

Sharding: tensor-parallel over heads across 8 cores. Core i gets Q heads
4i..4i+3 and KV head i. x is replicated (pre-transposed + fp16-cast on host).
Each core computes its 4 heads' attention and a partial output projection
(contraction over its 512 input dims of wo); the host sums the 8 partials.

Per-core dataflow (all matmuls fp16 with fp32 PSUM accumulation):

phase A (projections, dtile-major so the stationary weight tile is reused
across the full s=2048 moving range -> LDWEIGHTS amortized to ~3%):
  for dtile in [V, K, Q0..Q3]:
    psum[128, 2048] accumulated over 32 contraction blocks
      (per cb: one LDW + 4 x 512-wide matmuls)
    Q/K: rope applied on DVE into qt/kt [d, s] fp16 (partition-half pairing,
         sign-folded sin table -> 4 DVE passes)
    V: cast to fp16, then 16x DMA XBAR transpose into v_sb [s, d]
  x is resident in SBUF (16MB fp16), streamed in per-cb DMAs so the first
  dtile can start before the full load completes.

phase B (attention + oproj, chunk = 512 q columns, 2-head pairs so kt/v
stationary tiles serve 2 x 512-wide matmuls per LDW):
  per chunk t, head pair hp:
    for kb in 0..4(t+1):
      scores_T [k,q0|q1] one 2-bank psum tile; exp on ACT (scale folded);
      causal diag blocks masked by fp16 mask multiply on DVE (2x packed);
      dacc (softmax denominator partials) accumulated in fp16 on DVE (2x);
      PV accumulates attn_T [d, q0|q1] in psum
      (scores run one kb ahead of PV; o-proj matmul groups of the previous
       chunk are interleaved as PE filler under the ACT exp latency)
    denominator: ones-matmul partition-reduce of dacc -> reciprocal_approx_fast
    at = attn_T * recip -> fp16 [d, s] tiles
  o-proj per chunk: 4 s-blocks x 4 e-groups of 1024; at slice stationary,
  wo moving (1024 cycles per LDW); psum evacuated fp16, DMA per s-block.
Output is fp16 partials [S, DIM]; host sums 8 cores in fp32.
"""

import numpy as np

S = 2048
DIM = 4096
HD = 128
N_CORES = 8
QH_PER_CORE = 4  # 512 q dims per core
DQ = QH_PER_CORE * HD  # 512
SCALE = 1.0 / float(np.sqrt(HD))
SB = S // 128  # 16 s blocks
CB = DIM // 128  # 32 contraction blocks
NT = S // 512  # 4 q chunks
NEG = 4  # oproj e-groups of 1024 (DIM / 1024)
EXP_BIAS = -9.0  # exp(s*scale - 9): keeps fp16 pt finite (max score ~18.7)

f16 = np.float16

_RUNNER = None


def _build(reps=None):
    import concourse.bass as bass
    import concourse.mybir as mybir
    import concourse.tile as tile
    from concourse import bacc
    from contextlib import nullcontext

    dt = mybir.dt
    Exp = mybir.ActivationFunctionType.Exp

    nc = bacc.Bacc(
        "TRN2", target_bir_lowering=False, debug=False, num_devices=N_CORES
    )

    xt_d = nc.dram_tensor("xt", [DIM, S], dt.float16, kind="ExternalInput").ap()
    # weights pre-laid-out host-side as [c=128, dtile*4096] so the per-dtile
    # DMA reads 8KB/partition contiguously
    wt_d = nc.dram_tensor("wt", [128, 6 * CB * 128], dt.float16, kind="ExternalInput").ap()
    wot_d = nc.dram_tensor("wot", [DQ, DIM], dt.float16, kind="ExternalInput").ap()
    csd_d = nc.dram_tensor("csd", [128, S], dt.float16, kind="ExternalInput").ap()
    snd_d = nc.dram_tensor("snd", [128, S], dt.float16, kind="ExternalInput").ap()
    mask_d = nc.dram_tensor("mask", [128, 128], dt.float16, kind="ExternalInput").ap()
    out_d = nc.dram_tensor("out", [S, DIM], dt.float16, kind="ExternalOutput").ap()

    with tile.TileContext(nc) as tc:
        with tc.For_i(0, reps, 1) if reps else nullcontext(), tc.tile_pool(
            name="const", bufs=1
        ) as cp:
            mask_sb = cp.tile([128, 128], dt.float16)  # DMA'd at phase B start
            ones_sb = cp.tile([128, 128], dt.float16)
            nc.vector.memset(ones_sb, 1.0)
            ebias_sb = cp.tile([128, 1], dt.float32)
            nc.vector.memset(ebias_sb, EXP_BIAS)

            qt_sb = cp.tile([128, QH_PER_CORE, S], dt.float16)  # [d, h, s]
            kt_sb = cp.tile([128, S], dt.float16)  # [d, s]
            v_sb = cp.tile([128, SB, HD], dt.float16)  # [s128, sb, d]

            # ---------------- phase A: projections + rope ----------------
            with (
                tc.tile_pool(name="pa", bufs=1) as pa,
                tc.tile_pool(name="pap", bufs=2, space="PSUM") as pap,
            ):
                # the resident x stream + tables go on the scalar DMA queue;
                # weight tiles on the sync queue, so the first matmul only
                # waits for wt[V] + xt[cb0] (parallel queues).
                xt_sb = pa.tile([128, CB, S], dt.float16)
                xt_r = xt_d.rearrange("(cb c) s -> c cb s", c=128)
                for cb in range(CB):
                    nc.scalar.dma_start(out=xt_sb[:, cb, :], in_=xt_r[:, cb, :])
                csd_sb = pa.tile([128, S], dt.float16)
                nc.scalar.dma_start(out=csd_sb, in_=csd_d)
                snd_sb = pa.tile([128, S], dt.float16)  # sign-folded: -sin | +sin
                nc.scalar.dma_start(out=snd_sb, in_=snd_d)

                def rope_evac(ps, dest, pieces=2):
                    # ACT evacuates psum fast (frees the bank); a DMA swaps the
                    # partition halves (SBUF-only TT must be partition-aligned);
                    # rope then runs on DVE in all-fp16 (2x packed). Two
                    # pipelined s-halves shorten the tail after the last dtile.
                    # dest[0:64]   = a*cos - b*sin   (a = rows 0:64, b = rows 64:128)
                    # dest[64:128] = a*sin + b*cos
                    pw = S // pieces
                    for hh in range(pieces):
                        sl = slice(hh * pw, (hh + 1) * pw)
                        tmp = pa.tile([128, pw], dt.float16, tag="tmp", bufs=2,
                                      name=f"ropetmp_{hh % 2}")
                        nc.scalar.copy(tmp, ps[:, sl])
                        tsw = pa.tile([128, pw], dt.float16, tag="tsw", bufs=2,
                                      name=f"ropetsw_{hh % 2}")
                        nc.scalar.dma_start(out=tsw[0:64, :], in_=tmp[64:128, :])
                        nc.scalar.dma_start(out=tsw[64:128, :], in_=tmp[0:64, :])
                        tb = pa.tile([128, pw], dt.float16, tag="tb", bufs=2,
                                     name=f"ropetb_{hh % 2}")
                        nc.vector.tensor_mul(dest[:, sl], tmp, csd_sb[:, sl])
                        nc.vector.tensor_mul(tb, tsw, snd_sb[:, sl])
                        nc.vector.tensor_add(dest[:, sl], dest[:, sl], tb)

                def load_wt(dtile, name):
                    wt_sb = pa.tile(
                        [128, CB, 128], dt.float16, tag="wt", bufs=3, name=name
                    )
                    src = wt_d[:, dtile * 4096 : (dtile + 1) * 4096].rearrange(
                        "c (cb n) -> c cb n", n=128
                    )
                    nc.sync.dma_start(out=wt_sb[:, 0:4, :], in_=src[:, 0:4, :])
                    nc.sync.dma_start(out=wt_sb[:, 4:CB, :], in_=src[:, 4:CB, :])
                    return wt_sb

                def proj_mms(ps, wt_sb, cb):
                    for j in range(4):
                        nc.tensor.matmul(
                            ps[:, j * 512 : (j + 1) * 512],
                            lhsT=wt_sb[:, cb, :],
                            rhs=xt_sb[:, cb, j * 512 : (j + 1) * 512],
                            start=(cb == 0),
                            stop=(cb == CB - 1),
                        )

                # V and K share the first cb loop: 8 matmuls per xt slice keep
                # PE ahead of the x DMA stream (the stream is the startup
                # bottleneck); V's transposes then run mid-phase off-PE.
                wt_tiles = {d: load_wt(d, f"wt{d}") for d in (5, 4, 0, 1, 2, 3)}
                psV = pap.tile([128, S], dt.float32, tag="proj", name="psV")
                psK = pap.tile([128, S], dt.float32, tag="proj", name="psK")
                for cb in range(CB):
                    proj_mms(psV, wt_tiles[5], cb)
                    proj_mms(psK, wt_tiles[4], cb)
                vt_st = pa.tile([128, S], dt.float16, tag="vt")
                nc.scalar.copy(vt_st, psV)
                for b in range(SB):
                    nc.sync.dma_start_transpose(
                        out=v_sb[:, b, :],
                        in_=vt_st[:, b * 128 : (b + 1) * 128],
                    )
                rope_evac(psK, kt_sb)

                for dtile in (0, 1, 2, 3):
                    ps = pap.tile([128, S], dt.float32, tag="proj", name=f"psQ{dtile}")
                    for cb in range(CB):
                        proj_mms(ps, wt_tiles[dtile], cb)
                    rope_evac(ps, qt_sb[:, dtile, :])

            # ---------------- phase B: attention + output projection ------
            with (
                tc.tile_pool(name="pb", bufs=2) as pb,
                tc.tile_pool(name="pbp", bufs=1, space="PSUM") as pbp,
            ):
                woT_sb = pb.tile([128, QH_PER_CORE, DIM], dt.float16, bufs=1)
                woT_r = wot_d.rearrange("(db p) e -> p db e", p=128)
                for eh in range(4):  # split so early oproj e-groups start sooner
                    nc.sync.dma_start(
                        out=woT_sb[:, :, eh * 1024 : (eh + 1) * 1024],
                        in_=woT_r[:, :, eh * 1024 : (eh + 1) * 1024],
                    )
                nc.scalar.dma_start(out=mask_sb, in_=mask_d)

                def oproj_group(t, sbl, eg, ats, ptag="oproj"):
                    # out rows (4t+sbl)*128, e-columns eg*1024..+1024
                    ps_op = pbp.tile([128, 1024], dt.float32, tag=ptag,
                                     bufs=2 if ptag == "scores" else 1,
                                     name=f"psop_{t}_{sbl}_{eg}")
                    for hp in (0, 1):
                        for hh in (0, 1):
                            h = 2 * hp + hh
                            lhsT = ats[hp][
                                :, hh * 512 + sbl * 128 : hh * 512 + (sbl + 1) * 128
                            ]
                            for j in (0, 1):
                                nc.tensor.matmul(
                                    ps_op[:, j * 512 : (j + 1) * 512],
                                    lhsT=lhsT,
                                    rhs=woT_sb[
                                        :, h, eg * 1024 + j * 512 : eg * 1024 + (j + 1) * 512
                                    ],
                                    start=(h == 0),
                                    stop=(h == 3),
                                )
                    o_sb = osb_tiles[sbl]
                    ev = nc.vector.tensor_copy if eg % 2 == 0 else nc.scalar.copy
                    ev(o_sb[:, eg * 1024 : (eg + 1) * 1024], ps_op)
                    # per-eg DMAs so output transfers trickle out instead of
                    # piling up after the flush; all on the sync queue (idle in
                    # phase B) to keep triggers off the exp-loaded ACT engine
                    nc.sync.dma_start(
                        out=out_d[
                            (4 * t + sbl) * 128 : (4 * t + sbl + 1) * 128,
                            eg * 1024 : (eg + 1) * 1024,
                        ],
                        in_=o_sb[:, eg * 1024 : (eg + 1) * 1024],
                    )

                pending = []  # oproj closures for the previous chunk
                prev_ats = None
                prev_t = None
                # chunk 1 first (better ACT/PE balance with no oproj filler);
                # chunk 0 last, where chunk 3's oproj fills its short kb loops
                for t in (1, 2, 3, 0):
                    nkb = 4 * (t + 1)
                    if prev_ats is not None:
                        pats = prev_ats
                        pt_ = prev_t
                        pending = [
                            (pt_, sbl, eg, pats)
                            for sbl in range(4)
                            for eg in range(NEG)
                        ]
                        osb_tiles = [
                            pb.tile([128, DIM], dt.float16, name=f"osb_{t}_{sbl}",
                                    tag=f"osb{sbl}", bufs=1)
                            for sbl in range(4)
                        ]
                    at_tiles = []
                    for hp in (0, 1):
                        h0, h1 = 2 * hp, 2 * hp + 1
                        q0 = qt_sb[:, h0, t * 512 : (t + 1) * 512]
                        q1 = qt_sb[:, h1, t * 512 : (t + 1) * 512]
                        ps_o = pbp.tile([128, 1024], dt.float32, tag="pvo", bufs=1)
                        dacc = pb.tile([128, 1024], dt.float16, tag="dacc", bufs=2)
                        pt_tiles = [None] * nkb

                        def emit_scores(kb):
                            # diagonal blocks only need q >= k: skip the dead
                            # upper-triangle 128-col strips
                            lo = max(kb - 4 * t, 0) * 128
                            ps_s = pbp.tile(
                                [128, 1024], dt.float32, tag="scores", bufs=2
                            )
                            nc.tensor.matmul(
                                ps_s[:, lo:512],
                                lhsT=kt_sb[:, kb * 128 : (kb + 1) * 128],
                                rhs=q0[:, lo:512], start=True, stop=True,
                            )
                            nc.tensor.matmul(
                                ps_s[:, 512 + lo : 1024],
                                lhsT=kt_sb[:, kb * 128 : (kb + 1) * 128],
                                rhs=q1[:, lo:512], start=True, stop=True,
                            )
                            pt = pb.tile([128, 1024], dt.float16, tag="pt", bufs=4)
                            if lo == 0:
                                nc.scalar.activation(pt, ps_s, Exp, scale=SCALE, bias=ebias_sb)
                            else:
                                nc.scalar.activation(
                                    pt[:, lo:512], ps_s[:, lo:512],
                                    Exp, scale=SCALE, bias=ebias_sb,
                                )
                                nc.scalar.activation(
                                    pt[:, 512 + lo : 1024], ps_s[:, 512 + lo : 1024],
                                    Exp, scale=SCALE, bias=ebias_sb,
                                )
                            pt_tiles[kb] = pt

                        def emit_mask_dacc(kb):
                            pt = pt_tiles[kb]
                            j = kb - 4 * t
                            lo = max(j, 0) * 128
                            if j >= 0:  # triangular 128-block on the diagonal
                                nc.vector.tensor_mul(
                                    pt[:, lo : lo + 128], pt[:, lo : lo + 128], mask_sb
                                )
                                nc.vector.tensor_mul(
                                    pt[:, 512 + lo : 512 + lo + 128],
                                    pt[:, 512 + lo : 512 + lo + 128], mask_sb,
                                )
                            if kb == 0:
                                nc.vector.tensor_copy(dacc, pt)
                            elif lo == 0:
                                nc.vector.tensor_add(dacc, dacc, pt)
                            else:
                                nc.vector.tensor_add(
                                    dacc[:, lo:512], dacc[:, lo:512], pt[:, lo:512]
                                )
                                nc.vector.tensor_add(
                                    dacc[:, 512 + lo : 1024],
                                    dacc[:, 512 + lo : 1024], pt[:, 512 + lo : 1024],
                                )

                        emit_scores(0)
                        for kb in range(nkb):
                            if kb + 1 < nkb:
                                emit_scores(kb + 1)
                            emit_mask_dacc(kb)
                            pt = pt_tiles[kb]
                            lo = max(kb - 4 * t, 0) * 128
                            # 2 oproj groups at the hp boundary to cover the
                            # denominator->at chain of the previous hp
                            for _ in range(2 if kb == 0 else 1):
                                if pending:
                                    oproj_group(*pending.pop(0))
                            for j in (0, 1):
                                nc.tensor.matmul(
                                    ps_o[:, j * 512 + lo : (j + 1) * 512],
                                    lhsT=v_sb[:, kb, :],
                                    rhs=pt[:, j * 512 + lo : (j + 1) * 512],
                                    start=(kb == 0),
                                    stop=(kb == nkb - 1),
                                    skip_group_check=True,
                                )
                        # softmax denominator: every row of ps_d = column sum
                        ps_d = pbp.tile([128, 1024], dt.float32, tag="scores", bufs=2)
                        for j in (0, 1):
                            nc.tensor.matmul(
                                ps_d[:, j * 512 : (j + 1) * 512],
                                lhsT=ones_sb,
                                rhs=dacc[:, j * 512 : (j + 1) * 512],
                                start=True, stop=True,
                            )
                        recip = pb.tile([128, 1024], dt.float32, tag="recip")
                        nc.vector.reciprocal_approx_fast(recip, ps_d)
                        at = pb.tile([128, 1024], dt.float16, tag=f"at{hp}", bufs=2)
                        nc.vector.tensor_mul(at, ps_o, recip)
                        at_tiles.append(at)
                    while pending:
                        oproj_group(*pending.pop(0))
                    prev_ats = at_tiles
                    prev_t = t
                # flush last chunk's oproj
                osb_tiles = [
                    pb.tile([128, DIM], dt.float16, name=f"osb_f_{sbl}",
                            tag=f"osb{sbl}", bufs=1)
                    for sbl in range(4)
                ]
                # rotate psum tags so the flush triple-buffers (pvo and scores
                # are free once the last at-mul is done)
                ftags = ("oproj", "pvo", "scores")
                for i, (sbl, eg) in enumerate(
                    (s_, e_) for s_ in range(4) for e_ in range(NEG)
                ):
                    oproj_group(prev_t, sbl, eg, prev_ats, ptag=ftags[i % 3])
    nc.compile()
    return nc


def _prep_inputs(x, cos, sin, wq, wk, wv, wo):
    x = np.asarray(x, dtype=np.float32)
    cos = np.asarray(cos, dtype=np.float32)
    sin = np.asarray(sin, dtype=np.float32)
    wq = np.asarray(wq, dtype=np.float32)
    wk = np.asarray(wk, dtype=np.float32)
    wv = np.asarray(wv, dtype=np.float32)
    wo = np.asarray(wo, dtype=np.float32)

    xt = np.ascontiguousarray(x[0].T).astype(f16)  # [DIM, S]
    # cos/sin transposed, duplicated into both partition halves [128, S];
    # sin sign-folded: rows 0:64 = -sin (pairs with b), rows 64:128 = +sin
    csd = np.tile(cos.T, (2, 1)).astype(f16)
    snd = np.concatenate([-sin.T, sin.T], axis=0).astype(f16)
    # de-interleave perm: head dim pairs (2i, 2i+1) -> rows (i, 64+i)
    perm = np.concatenate([np.arange(0, HD, 2), np.arange(1, HD, 2)])

    # triangular mask for the diagonal 128x128 sub-block: keep k <= q
    r = np.arange(128)[:, None]
    c = np.arange(128)[None, :]
    mask = (r <= c).astype(f16)  # [128, 128]

    in_maps = []
    for i in range(N_CORES):
        wq_i = wq[DQ * i : DQ * (i + 1)]  # [512, DIM]
        wk_i = wk[HD * i : HD * (i + 1)]  # [128, DIM]
        wv_i = wv[HD * i : HD * (i + 1)]
        wq_p = wq_i.reshape(QH_PER_CORE, HD, DIM)[:, perm, :].reshape(DQ, DIM)
        wk_p = wk_i[perm, :]
        w_all = np.concatenate([wq_p, wk_p, wv_i], axis=0)  # [768, DIM]
        # layout [c=128, dtile, cb, n=128]: element (dtile*128+n, cb*128+c)
        wt = (
            w_all.reshape(6, 128, CB, 128)  # [dtile, n, cb, c]
            .transpose(3, 0, 2, 1)  # [c, dtile, cb, n]
            .reshape(128, 6 * CB * 128)
            .astype(f16)
        )
        wot = np.ascontiguousarray(wo[:, DQ * i : DQ * (i + 1)].T).astype(f16)
        in_maps.append(
            {
                "xt": xt,
                "wt": np.ascontiguousarray(wt),
                "wot": wot,
                "csd": np.ascontiguousarray(csd),
                "snd": np.ascontiguousarray(snd),
                "mask": np.ascontiguousarray(mask),
            }
        )
    return in_maps


def _get_runner():
    global _RUNNER
    if _RUNNER is None:
        _RUNNER = _build()
    return _RUNNER


def kernel(x, cos, sin, wq, wk, wv, wo):
    from concourse.bass_utils import run_bass_kernel_spmd

    nc = _get_runner()
    in_maps = _prep_inputs(x, cos, sin, wq, wk, wv, wo)
    res = run_bass_kernel_spmd(nc, in_maps, list(range(N_CORES)))
    out = np.zeros((S, DIM), dtype=np.float32)
    for i in range(N_CORES):
        out += res.results[i]["out"].astype(np.float32)
    return out[None].astype(np.float32)


# revision 51
# speedup vs baseline: 1.0145x; 1.0145x over previous
"""Trainium2 Bass kernel for GQA attention prefill (Mistral-style, RoPE, causal).

B=1, S=2048, DIM=4096, 32 Q heads / 8 KV heads, HD=128, rope theta 1e6.

Sharding: tensor-parallel over heads across 8 cores. Core i gets Q heads
4i..4i+3 and KV head i. x is replicated (pre-transposed + fp16-cast on host).
Each core computes its 4 heads' attention and a partial output projection
(contraction over its 512 input dims of wo); the host sums the 8 partials.

Per-core dataflow (all matmuls fp16 with fp32 PSUM accumulation):

phase A (projections, dtile-major so the stationary weight tile is reused
across the full s=2048 moving range -> LDWEIGHTS amortized to ~3%):
  for dtile in [V, K, Q0..Q3]:
    psum[128, 2048] accumulated over 32 contraction blocks
      (per cb: one LDW + 4 x 512-wide matmuls)
    Q/K: rope applied on DVE into qt/kt [d, s] fp16 (partition-half pairing,
         sign-folded sin table -> 4 DVE passes)
    V: cast to fp16, then 16x DMA XBAR transpose into v_sb [s, d]
  x is resident in SBUF (16MB fp16), streamed in per-cb DMAs so the first
  dtile can start before the full load completes.

phase B (attention + oproj, chunk = 512 q columns, 2-head pairs so kt/v
stationary tiles serve 2 x 512-wide matmuls per LDW):
  per chunk t, head pair hp:
    for kb in 0..4(t+1):
      scores_T [k,q0|q1] one 2-bank psum tile; exp on ACT (scale folded);
      causal diag blocks masked by fp16 mask multiply on DVE (2x packed);
      dacc (softmax denominator partials) accumulated in fp16 on DVE (2x);
      PV accumulates attn_T [d, q0|q1] in psum
      (scores run one kb ahead of PV; o-proj matmul groups of the previous
       chunk are interleaved as PE filler under the ACT exp latency)
    denominator: ones-matmul partition-reduce of dacc -> reciprocal_approx_fast
    at = attn_T * recip -> fp16 [d, s] tiles
  o-proj per chunk: 4 s-blocks x 4 e-groups of 1024; at slice stationary,
  wo moving (1024 cycles per LDW); psum evacuated fp16, DMA per s-block.
Output is fp16 partials [S, DIM]; host sums 8 cores in fp32.
"""

import numpy as np

S = 2048
DIM = 4096
HD = 128
N_CORES = 8
QH_PER_CORE = 4  # 512 q dims per core
DQ = QH_PER_CORE * HD  # 512
SCALE = 1.0 / float(np.sqrt(HD))
SB = S // 128  # 16 s blocks
CB = DIM // 128  # 32 contraction blocks
NT = S // 512  # 4 q chunks
NEG = 4  # oproj e-groups of 1024 (DIM / 1024)
EXP_BIAS = -9.0  # exp(s*scale - 9): keeps fp16 pt finite (max score ~18.7)

f16 = np.float16

_RUNNER = None


def _build(reps=None):
    import concourse.bass as bass
    import concourse.mybir as mybir
    import concourse.tile as tile
    from concourse import bacc
    from contextlib import nullcontext

    dt = mybir.dt
    Exp = mybir.ActivationFunctionType.Exp

    nc = bacc.Bacc(
        "TRN2", target_bir_lowering=False, debug=False, num_devices=N_CORES
    )

    xt_d = nc.dram_tensor("xt", [DIM, S], dt.float16, kind="ExternalInput").ap()
    # weights pre-laid-out host-side as [c=128, dtile*4096] so the per-dtile
    # DMA reads 8KB/partition contiguously
    wt_d = nc.dram_tensor("wt", [128, 6 * CB * 128], dt.float16, kind="ExternalInput").ap()
    wot_d = nc.dram_tensor("wot", [DQ, DIM], dt.float16, kind="ExternalInput").ap()
    csd_d = nc.dram_tensor("csd", [128, S], dt.float16, kind="ExternalInput").ap()
    snd_d = nc.dram_tensor("snd", [128, S], dt.float16, kind="ExternalInput").ap()
    mask_d = nc.dram_tensor("mask", [128, 128], dt.float16, kind="ExternalInput").ap()
    out_d = nc.dram_tensor("out", [S, DIM], dt.float16, kind="ExternalOutput").ap()

    with tile.TileContext(nc) as tc:
        with tc.For_i(0, reps, 1) if reps else nullcontext(), tc.tile_pool(
            name="const", bufs=1
        ) as cp:
            mask_sb = cp.tile([128, 128], dt.float16)  # DMA'd at phase B start
            ones_sb = cp.tile([128, 128], dt.float16)
            nc.vector.memset(ones_sb, 1.0)
            ebias_sb = cp.tile([128, 1], dt.float32)
            nc.vector.memset(ebias_sb, EXP_BIAS)

            qt_sb = cp.tile([128, QH_PER_CORE, S], dt.float16)  # [d, h, s]
            kt_sb = cp.tile([128, S], dt.float16)  # [d, s]
            v_sb = cp.tile([128, SB, HD], dt.float16)  # [s128, sb, d]

            # ---------------- phase A: projections + rope ----------------
            with (
                tc.tile_pool(name="pa", bufs=1) as pa,
                tc.tile_pool(name="pap", bufs=2, space="PSUM") as pap,
            ):
                # the resident x stream + tables go on the scalar DMA queue;
                # weight tiles on the sync queue, so the first matmul only
                # waits for wt[V] + xt[cb0] (parallel queues).
                xt_sb = pa.tile([128, CB, S], dt.float16)
                xt_r = xt_d.rearrange("(cb c) s -> c cb s", c=128)
                for cb in range(CB):
                    nc.scalar.dma_start(out=xt_sb[:, cb, :], in_=xt_r[:, cb, :])
                csd_sb = pa.tile([128, S], dt.float16)
                nc.scalar.dma_start(out=csd_sb, in_=csd_d)
                snd_sb = pa.tile([128, S], dt.float16)  # sign-folded: -sin | +sin
                nc.scalar.dma_start(out=snd_sb, in_=snd_d)

                def rope_evac(ps, dest, pieces=2):
                    # ACT evacuates psum fast (frees the bank); a DMA swaps the
                    # partition halves (SBUF-only TT must be partition-aligned);
                    # rope then runs on DVE in all-fp16 (2x packed). Two
                    # pipelined s-halves shorten the tail after the last dtile.
                    # dest[0:64]   = a*cos - b*sin   (a = rows 0:64, b = rows 64:128)
                    # dest[64:128] = a*sin + b*cos
                    pw = S // pieces
                    for hh in range(pieces):
                        sl = slice(hh * pw, (hh + 1) * pw)
                        tmp = pa.tile([128, pw], dt.float16, tag="tmp", bufs=2,
                                      name=f"ropetmp_{hh % 2}")
                        nc.scalar.copy(tmp, ps[:, sl])
                        tsw = pa.tile([128, pw], dt.float16, tag="tsw", bufs=2,
                                      name=f"ropetsw_{hh % 2}")
                        nc.scalar.dma_start(out=tsw[0:64, :], in_=tmp[64:128, :])
                        nc.scalar.dma_start(out=tsw[64:128, :], in_=tmp[0:64, :])
                        tb = pa.tile([128, pw], dt.float16, tag="tb", bufs=2,
                                     name=f"ropetb_{hh % 2}")
                        nc.vector.tensor_mul(dest[:, sl], tmp, csd_sb[:, sl])
                        nc.vector.tensor_mul(tb, tsw, snd_sb[:, sl])
                        nc.vector.tensor_add(dest[:, sl], dest[:, sl], tb)

                def load_wt(dtile, name):
                    wt_sb = pa.tile(
                        [128, CB, 128], dt.float16, tag="wt", bufs=2, name=name
                    )
                    src = wt_d[:, dtile * 4096 : (dtile + 1) * 4096].rearrange(
                        "c (cb n) -> c cb n", n=128
                    )
                    nc.sync.dma_start(out=wt_sb[:, 0:4, :], in_=src[:, 0:4, :])
                    nc.sync.dma_start(out=wt_sb[:, 4:CB, :], in_=src[:, 4:CB, :])
                    return wt_sb

                def proj_mms(ps, wt_sb, cb):
                    for j in range(4):
                        nc.tensor.matmul(
                            ps[:, j * 512 : (j + 1) * 512],
                            lhsT=wt_sb[:, cb, :],
                            rhs=xt_sb[:, cb, j * 512 : (j + 1) * 512],
                            start=(cb == 0),
                            stop=(cb == CB - 1),
                        )

                # V and K share the first cb loop: 8 matmuls per xt slice keep
                # PE ahead of the x DMA stream (the stream is the startup
                # bottleneck); V's transposes then run mid-phase off-PE.
                wt_tiles = {d: load_wt(d, f"wt{d}") for d in (5, 4, 0, 1, 2, 3)}
                psV = pap.tile([128, S], dt.float32, tag="proj", name="psV")
                psK = pap.tile([128, S], dt.float32, tag="proj", name="psK")
                for cb in range(CB):
                    proj_mms(psV, wt_tiles[5], cb)
                    proj_mms(psK, wt_tiles[4], cb)
                vt_st = pa.tile([128, S], dt.float16, tag="vt")
                nc.scalar.copy(vt_st, psV)
                for b in range(SB):
                    nc.sync.dma_start_transpose(
                        out=v_sb[:, b, :],
                        in_=vt_st[:, b * 128 : (b + 1) * 128],
                    )
                rope_evac(psK, kt_sb)

                for dtile in (0, 1, 2, 3):
                    ps = pap.tile([128, S], dt.float32, tag="proj", name=f"psQ{dtile}")
                    for cb in range(CB):
                        proj_mms(ps, wt_tiles[dtile], cb)
                    rope_evac(ps, qt_sb[:, dtile, :])

            # ---------------- phase B: attention + output projection ------
            with (
                tc.tile_pool(name="pb", bufs=2) as pb,
                tc.tile_pool(name="pbp", bufs=1, space="PSUM") as pbp,
            ):
                woT_sb = pb.tile([128, QH_PER_CORE, DIM], dt.float16, bufs=1)
                woT_r = wot_d.rearrange("(db p) e -> p db e", p=128)
                for eh in range(4):  # split so early oproj e-groups start sooner
                    nc.sync.dma_start(
                        out=woT_sb[:, :, eh * 1024 : (eh + 1) * 1024],
                        in_=woT_r[:, :, eh * 1024 : (eh + 1) * 1024],
                    )
                nc.scalar.dma_start(out=mask_sb, in_=mask_d)

                def oproj_group(t, sbl, eg, ats, ptag="oproj"):
                    # out rows (4t+sbl)*128, e-columns eg*1024..+1024
                    ps_op = pbp.tile([128, 1024], dt.float32, tag=ptag,
                                     bufs=2 if ptag == "scores" else 1,
                                     name=f"psop_{t}_{sbl}_{eg}")
                    for hp in (0, 1):
                        for hh in (0, 1):
                            h = 2 * hp + hh
                            lhsT = ats[hp][
                                :, hh * 512 + sbl * 128 : hh * 512 + (sbl + 1) * 128
                            ]
                            for j in (0, 1):
                                nc.tensor.matmul(
                                    ps_op[:, j * 512 : (j + 1) * 512],
                                    lhsT=lhsT,
                                    rhs=woT_sb[
                                        :, h, eg * 1024 + j * 512 : eg * 1024 + (j + 1) * 512
                                    ],
                                    start=(h == 0),
                                    stop=(h == 3),
                                )
                    o_sb = osb_tiles[sbl]
                    ev = nc.vector.tensor_copy if eg % 2 == 0 else nc.scalar.copy
                    ev(o_sb[:, eg * 1024 : (eg + 1) * 1024], ps_op)
                    # per-eg DMAs so output transfers trickle out instead of
                    # piling up after the flush; all on the sync queue (idle in
                    # phase B) to keep triggers off the exp-loaded ACT engine
                    nc.sync.dma_start(
                        out=out_d[
                            (4 * t + sbl) * 128 : (4 * t + sbl + 1) * 128,
                            eg * 1024 : (eg + 1) * 1024,
                        ],
                        in_=o_sb[:, eg * 1024 : (eg + 1) * 1024],
                    )

                pending = []  # oproj closures for the previous chunk
                prev_ats = None
                prev_t = None
                # chunk 1 first (better ACT/PE balance with no oproj filler);
                # chunk 0 last, where chunk 3's oproj fills its short kb loops
                for t in (1, 2, 3, 0):
                    nkb = 4 * (t + 1)
                    if prev_ats is not None:
                        pats = prev_ats
                        pt_ = prev_t
                        pending = [
                            (pt_, sbl, eg, pats)
                            for sbl in range(4)
                            for eg in range(NEG)
                        ]
                        osb_tiles = [
                            pb.tile([128, DIM], dt.float16, name=f"osb_{t}_{sbl}",
                                    tag=f"osb{sbl}", bufs=1)
                            for sbl in range(4)
                        ]
                    at_tiles = []
                    for hp in (0, 1):
                        h0, h1 = 2 * hp, 2 * hp + 1
                        q0 = qt_sb[:, h0, t * 512 : (t + 1) * 512]
                        q1 = qt_sb[:, h1, t * 512 : (t + 1) * 512]
                        ps_o = pbp.tile([128, 1024], dt.float32, tag="pvo", bufs=1)
                        # cp (outer) pool: no address overlap with phase A's
                        # SBUF, so chunk-1 work starts before the pa pool closes
                        dacc = cp.tile([128, 1024], dt.float16, tag="dacc", bufs=2,
                                       name=f"dacc_{t}_{hp}")
                        pt_tiles = [None] * nkb

                        def emit_scores(kb):
                            # diagonal blocks only need q >= k: skip the dead
                            # upper-triangle 128-col strips
                            lo = max(kb - 4 * t, 0) * 128
                            ps_s = pbp.tile(
                                [128, 1024], dt.float32, tag="scores", bufs=2
                            )
                            nc.tensor.matmul(
                                ps_s[:, lo:512],
                                lhsT=kt_sb[:, kb * 128 : (kb + 1) * 128],
                                rhs=q0[:, lo:512], start=True, stop=True,
                            )
                            nc.tensor.matmul(
                                ps_s[:, 512 + lo : 1024],
                                lhsT=kt_sb[:, kb * 128 : (kb + 1) * 128],
                                rhs=q1[:, lo:512], start=True, stop=True,
                            )
                            pt = cp.tile([128, 1024], dt.float16, tag="pt", bufs=4,
                                         name=f"pt_{t}_{hp}_{kb}")
                            if lo == 0:
                                nc.scalar.activation(pt, ps_s, Exp, scale=SCALE, bias=ebias_sb)
                            else:
                                nc.scalar.activation(
                                    pt[:, lo:512], ps_s[:, lo:512],
                                    Exp, scale=SCALE, bias=ebias_sb,
                                )
                                nc.scalar.activation(
                                    pt[:, 512 + lo : 1024], ps_s[:, 512 + lo : 1024],
                                    Exp, scale=SCALE, bias=ebias_sb,
                                )
                            pt_tiles[kb] = pt

                        def emit_mask_dacc(kb):
                            pt = pt_tiles[kb]
                            j = kb - 4 * t
                            lo = max(j, 0) * 128
                            if j >= 0:  # triangular 128-block on the diagonal
                                nc.vector.tensor_mul(
                                    pt[:, lo : lo + 128], pt[:, lo : lo + 128], mask_sb
                                )
                                nc.vector.tensor_mul(
                                    pt[:, 512 + lo : 512 + lo + 128],
                                    pt[:, 512 + lo : 512 + lo + 128], mask_sb,
                                )
                            if kb == 0:
                                nc.vector.tensor_copy(dacc, pt)
                            elif lo == 0:
                                nc.vector.tensor_add(dacc, dacc, pt)
                            else:
                                nc.vector.tensor_add(
                                    dacc[:, lo:512], dacc[:, lo:512], pt[:, lo:512]
                                )
                                nc.vector.tensor_add(
                                    dacc[:, 512 + lo : 1024],
                                    dacc[:, 512 + lo : 1024], pt[:, 512 + lo : 1024],
                                )

                        emit_scores(0)
                        for kb in range(nkb):
                            if kb + 1 < nkb:
                                emit_scores(kb + 1)
                            emit_mask_dacc(kb)
                            pt = pt_tiles[kb]
                            lo = max(kb - 4 * t, 0) * 128
                            # 2 oproj groups at the hp boundary to cover the
                            # denominator->at chain of the previous hp
                            for _ in range(2 if kb == 0 else 1):
                                if pending:
                                    oproj_group(*pending.pop(0))
                            for j in (0, 1):
                                nc.tensor.matmul(
                                    ps_o[:, j * 512 + lo : (j + 1) * 512],
                                    lhsT=v_sb[:, kb, :],
                                    rhs=pt[:, j * 512 + lo : (j + 1) * 512],
                                    start=(kb == 0),
                                    stop=(kb == nkb - 1),
                                    skip_group_check=True,
                                )
                        # softmax denominator: every row of ps_d = column sum
                        ps_d = pbp.tile([128, 1024], dt.float32, tag="scores", bufs=2)
                        for j in (0, 1):
                            nc.tensor.matmul(
                                ps_d[:, j * 512 : (j + 1) * 512],
                                lhsT=ones_sb,
                                rhs=dacc[:, j * 512 : (j + 1) * 512],
                                start=True, stop=True,
                            )
                        recip = pb.tile([128, 1024], dt.float32, tag="recip")
                        nc.vector.reciprocal_approx_fast(recip, ps_d)
                        at = pb.tile([128, 1024], dt.float16, tag=f"at{hp}", bufs=2)
                        nc.vector.tensor_mul(at, ps_o, recip)
                        at_tiles.append(at)
                    while pending:
                        oproj_group(*pending.pop(0))
                    prev_ats = at_tiles
                    prev_t = t
                # flush last chunk's oproj
                osb_tiles = [
                    pb.tile([128, DIM], dt.float16, name=f"osb_f_{sbl}",
                            tag=f"osb{sbl}", bufs=1)
                    for sbl in range(4)
                ]
                # rotate psum tags so the flush triple-buffers (pvo and scores
                # are free once the last at-mul is done)
                ftags = ("oproj", "pvo", "scores")
                for i, (sbl, eg) in enumerate(
                    (s_, e_) for s_ in range(4) for e_ in range(NEG)
                ):
                    oproj_group(prev_t, sbl, eg, prev_ats, ptag=ftags[i % 3])
    nc.compile()
    return nc


def _prep_inputs(x, cos, sin, wq, wk, wv, wo):
    x = np.asarray(x, dtype=np.float32)
    cos = np.asarray(cos, dtype=np.float32)
    sin = np.asarray(sin, dtype=np.float32)
    wq = np.asarray(wq, dtype=np.float32)
    wk = np.asarray(wk, dtype=np.float32)
    wv = np.asarray(wv, dtype=np.float32)
    wo = np.asarray(wo, dtype=np.float32)

    xt = np.ascontiguousarray(x[0].T).astype(f16)  # [DIM, S]
    # cos/sin transposed, duplicated into both partition halves [128, S];
    # sin sign-folded: rows 0:64 = -sin (pairs with b), rows 64:128 = +sin
    csd = np.tile(cos.T, (2, 1)).astype(f16)
    snd = np.concatenate([-sin.T, sin.T], axis=0).astype(f16)
    # de-interleave perm: head dim pairs (2i, 2i+1) -> rows (i, 64+i)
    perm = np.concatenate([np.arange(0, HD, 2), np.arange(1, HD, 2)])

    # triangular mask for the diagonal 128x128 sub-block: keep k <= q
    r = np.arange(128)[:, None]
    c = np.arange(128)[None, :]
    mask = (r <= c).astype(f16)  # [128, 128]

    in_maps = []
    for i in range(N_CORES):
        wq_i = wq[DQ * i : DQ * (i + 1)]  # [512, DIM]
        wk_i = wk[HD * i : HD * (i + 1)]  # [128, DIM]
        wv_i = wv[HD * i : HD * (i + 1)]
        wq_p = wq_i.reshape(QH_PER_CORE, HD, DIM)[:, perm, :].reshape(DQ, DIM)
        wk_p = wk_i[perm, :]
        w_all = np.concatenate([wq_p, wk_p, wv_i], axis=0)  # [768, DIM]
        # layout [c=128, dtile, cb, n=128]: element (dtile*128+n, cb*128+c)
        wt = (
            w_all.reshape(6, 128, CB, 128)  # [dtile, n, cb, c]
            .transpose(3, 0, 2, 1)  # [c, dtile, cb, n]
            .reshape(128, 6 * CB * 128)
            .astype(f16)
        )
        wot = np.ascontiguousarray(wo[:, DQ * i : DQ * (i + 1)].T).astype(f16)
        in_maps.append(
            {
                "xt": xt,
                "wt": np.ascontiguousarray(wt),
                "wot": wot,
                "csd": np.ascontiguousarray(csd),
                "snd": np.ascontiguousarray(snd),
                "mask": np.ascontiguousarray(mask),
            }
        )
    return in_maps


def _get_runner():
    global _RUNNER
    if _RUNNER is None:
        _RUNNER = _build()
    return _RUNNER


def kernel(x, cos, sin, wq, wk, wv, wo):
    from concourse.bass_utils import run_bass_kernel_spmd

    nc = _get_runner()
    in_maps = _prep_inputs(x, cos, sin, wq, wk, wv, wo)
    res = run_bass_kernel_spmd(nc, in_maps, list(range(N_CORES)))
    out = np.zeros((S, DIM), dtype=np.float32)
    for i in range(N_CORES):
        out += res.results[i]["out"].astype(np.float32)
    return out[None].astype(np.float32)


# revision 55
# speedup vs baseline: 1.0146x; 1.0001x over previous
"""Trainium2 Bass kernel for GQA attention prefill (Mistral-style, RoPE, causal).

B=1, S=2048, DIM=4096, 32 Q heads / 8 KV heads, HD=128, rope theta 1e6.

Sharding: tensor-parallel over heads across 8 cores. Core i gets Q heads
4i..4i+3 and KV head i. x is replicated (pre-transposed + fp16-cast on host).
Each core computes its 4 heads' attention and a partial output projection
(contraction over its 512 input dims of wo); the host sums the 8 partials.

Per-core dataflow (all matmuls fp16 with fp32 PSUM accumulation):

phase A (projections, dtile-major so the stationary weight tile is reused
across the full s=2048 moving range -> LDWEIGHTS amortized to ~3%):
  for dtile in [V, K, Q0..Q3]:
    psum[128, 2048] accumulated over 32 contraction blocks
      (per cb: one LDW + 4 x 512-wide matmuls)
    Q/K: rope applied on DVE into qt/kt [d, s] fp16 (partition-half pairing,
         sign-folded sin table -> 4 DVE passes)
    V: cast to fp16, then 16x DMA XBAR transpose into v_sb [s, d]
  x is resident in SBUF (16MB fp16), streamed in per-cb DMAs so the first
  dtile can start before the full load completes.

phase B (attention + oproj, chunk = 512 q columns, 2-head pairs so kt/v
stationary tiles serve 2 x 512-wide matmuls per LDW):
  per chunk t, head pair hp:
    for kb in 0..4(t+1):
      scores_T [k,q0|q1] one 2-bank psum tile; exp on ACT (scale folded);
      causal diag blocks masked by fp16 mask multiply on DVE (2x packed);
      dacc (softmax denominator partials) accumulated in fp16 on DVE (2x);
      PV accumulates attn_T [d, q0|q1] in psum
      (scores run one kb ahead of PV; o-proj matmul groups of the previous
       chunk are interleaved as PE filler under the ACT exp latency)
    denominator: ones-matmul partition-reduce of dacc -> reciprocal_approx_fast
    at = attn_T * recip -> fp16 [d, s] tiles
  o-proj per chunk: 4 s-blocks x 4 e-groups of 1024; at slice stationary,
  wo moving (1024 cycles per LDW); psum evacuated fp16, DMA per s-block.
Output is fp16 partials [S, DIM]; host sums 8 cores in fp32.
"""

import numpy as np

S = 2048
DIM = 4096
HD = 128
N_CORES = 8
QH_PER_CORE = 4  # 512 q dims per core
DQ = QH_PER_CORE * HD  # 512
SCALE = 1.0 / float(np.sqrt(HD))
SB = S // 128  # 16 s blocks
CB = DIM // 128  # 32 contraction blocks
NT = S // 512  # 4 q chunks
NEG = 4  # oproj e-groups of 1024 (DIM / 1024)
EXP_BIAS = -9.0  # exp(s*scale - 9): keeps fp16 pt finite (max score ~18.7)

f16 = np.float16

_RUNNER = None


def _build(reps=None):
    import concourse.bass as bass
    import concourse.mybir as mybir
    import concourse.tile as tile
    from concourse import bacc
    from contextlib import nullcontext

    dt = mybir.dt
    Exp = mybir.ActivationFunctionType.Exp

    nc = bacc.Bacc(
        "TRN2", target_bir_lowering=False, debug=False, num_devices=N_CORES
    )

    xt_d = nc.dram_tensor("xt", [DIM, S], dt.float16, kind="ExternalInput").ap()
    # weights pre-laid-out host-side as [c=128, dtile*4096] so the per-dtile
    # DMA reads 8KB/partition contiguously
    wt_d = nc.dram_tensor("wt", [128, 6 * CB * 128], dt.float16, kind="ExternalInput").ap()
    wot_d = nc.dram_tensor("wot", [DQ, DIM], dt.float16, kind="ExternalInput").ap()
    csd_d = nc.dram_tensor("csd", [128, S], dt.float16, kind="ExternalInput").ap()
    snd_d = nc.dram_tensor("snd", [128, S], dt.float16, kind="ExternalInput").ap()
    mask_d = nc.dram_tensor("mask", [128, 128], dt.float16, kind="ExternalInput").ap()
    out_d = nc.dram_tensor("out", [S, DIM], dt.float16, kind="ExternalOutput").ap()

    with tile.TileContext(nc) as tc:
        with tc.For_i(0, reps, 1) if reps else nullcontext(), tc.tile_pool(
            name="const", bufs=1
        ) as cp:
            mask_sb = cp.tile([128, 128], dt.float16)  # DMA'd at phase B start
            ones_sb = cp.tile([128, 128], dt.float16)
            nc.vector.memset(ones_sb, 1.0)
            ebias_sb = cp.tile([128, 1], dt.float32)
            nc.vector.memset(ebias_sb, EXP_BIAS)

            qt_sb = cp.tile([128, QH_PER_CORE, S], dt.float16)  # [d, h, s]
            kt_sb = cp.tile([128, S], dt.float16)  # [d, s]
            v_sb = cp.tile([128, SB, HD], dt.float16)  # [s128, sb, d]

            # ---------------- phase A: projections + rope ----------------
            with (
                tc.tile_pool(name="pa", bufs=1) as pa,
                tc.tile_pool(name="pap", bufs=2, space="PSUM") as pap,
            ):
                # the resident x stream + tables go on the scalar DMA queue;
                # weight tiles on the sync queue, so the first matmul only
                # waits for wt[V] + xt[cb0] (parallel queues).
                xt_sb = pa.tile([128, CB, S], dt.float16)
                xt_r = xt_d.rearrange("(cb c) s -> c cb s", c=128)
                for cb in range(CB):
                    nc.scalar.dma_start(out=xt_sb[:, cb, :], in_=xt_r[:, cb, :])
                csd_sb = pa.tile([128, S], dt.float16)
                nc.scalar.dma_start(out=csd_sb, in_=csd_d)
                snd_sb = pa.tile([128, S], dt.float16)  # sign-folded: -sin | +sin
                nc.scalar.dma_start(out=snd_sb, in_=snd_d)

                def rope_evac(ps, dest, pieces=2):
                    # ACT evacuates psum fast (frees the bank); a DMA swaps the
                    # partition halves (SBUF-only TT must be partition-aligned);
                    # rope then runs on DVE in all-fp16 (2x packed). Two
                    # pipelined s-halves shorten the tail after the last dtile.
                    # dest[0:64]   = a*cos - b*sin   (a = rows 0:64, b = rows 64:128)
                    # dest[64:128] = a*sin + b*cos
                    pw = S // pieces
                    for hh in range(pieces):
                        sl = slice(hh * pw, (hh + 1) * pw)
                        tmp = pa.tile([128, pw], dt.float16, tag="tmp", bufs=2,
                                      name=f"ropetmp_{hh % 2}")
                        nc.scalar.copy(tmp, ps[:, sl])
                        tsw = pa.tile([128, pw], dt.float16, tag="tsw", bufs=2,
                                      name=f"ropetsw_{hh % 2}")
                        nc.scalar.dma_start(out=tsw[0:64, :], in_=tmp[64:128, :])
                        nc.scalar.dma_start(out=tsw[64:128, :], in_=tmp[0:64, :])
                        tb = pa.tile([128, pw], dt.float16, tag="tb", bufs=2,
                                     name=f"ropetb_{hh % 2}")
                        nc.vector.tensor_mul(dest[:, sl], tmp, csd_sb[:, sl])
                        nc.vector.tensor_mul(tb, tsw, snd_sb[:, sl])
                        nc.vector.tensor_add(dest[:, sl], dest[:, sl], tb)

                def load_wt(dtile, name):
                    wt_sb = pa.tile(
                        [128, CB, 128], dt.float16, tag="wt", bufs=2, name=name
                    )
                    src = wt_d[:, dtile * 4096 : (dtile + 1) * 4096].rearrange(
                        "c (cb n) -> c cb n", n=128
                    )
                    nc.sync.dma_start(out=wt_sb[:, 0:4, :], in_=src[:, 0:4, :])
                    nc.sync.dma_start(out=wt_sb[:, 4:CB, :], in_=src[:, 4:CB, :])
                    return wt_sb

                def proj_mms(ps, wt_sb, cb):
                    for j in range(4):
                        nc.tensor.matmul(
                            ps[:, j * 512 : (j + 1) * 512],
                            lhsT=wt_sb[:, cb, :],
                            rhs=xt_sb[:, cb, j * 512 : (j + 1) * 512],
                            start=(cb == 0),
                            stop=(cb == CB - 1),
                        )

                # V and K share the first cb loop: 8 matmuls per xt slice keep
                # PE ahead of the x DMA stream (the stream is the startup
                # bottleneck); V's transposes then run mid-phase off-PE.
                wt_tiles = {d: load_wt(d, f"wt{d}") for d in (5, 4, 0, 1, 2, 3)}
                psV = pap.tile([128, S], dt.float32, tag="proj", name="psV")
                psK = pap.tile([128, S], dt.float32, tag="proj", name="psK")
                for cb in range(CB):
                    proj_mms(psV, wt_tiles[5], cb)
                    proj_mms(psK, wt_tiles[4], cb)
                vt_st = pa.tile([128, S], dt.float16, tag="vt")
                nc.scalar.copy(vt_st, psV)
                for b in range(SB):
                    nc.sync.dma_start_transpose(
                        out=v_sb[:, b, :],
                        in_=vt_st[:, b * 128 : (b + 1) * 128],
                    )
                rope_evac(psK, kt_sb)

                for dtile in (0, 1, 2, 3):
                    ps = pap.tile([128, S], dt.float32, tag="proj", name=f"psQ{dtile}")
                    for cb in range(CB):
                        proj_mms(ps, wt_tiles[dtile], cb)
                    rope_evac(ps, qt_sb[:, dtile, :])

            # ---------------- phase B: attention + output projection ------
            with (
                tc.tile_pool(name="pb", bufs=2) as pb,
                tc.tile_pool(name="pbp", bufs=1, space="PSUM") as pbp,
            ):
                woT_sb = pb.tile([128, QH_PER_CORE, DIM], dt.float16, bufs=1)
                woT_r = wot_d.rearrange("(db p) e -> p db e", p=128)
                for eh in range(4):  # split so early oproj e-groups start sooner
                    nc.sync.dma_start(
                        out=woT_sb[:, :, eh * 1024 : (eh + 1) * 1024],
                        in_=woT_r[:, :, eh * 1024 : (eh + 1) * 1024],
                    )
                nc.scalar.dma_start(out=mask_sb, in_=mask_d)

                def oproj_group(t, sbl, eg, ats, ptag="oproj", ev_dve=False):
                    # out rows (4t+sbl)*128, e-columns eg*1024..+1024
                    ps_op = pbp.tile([128, 1024], dt.float32, tag=ptag,
                                     bufs=2 if ptag == "scores" else 1,
                                     name=f"psop_{t}_{sbl}_{eg}")
                    for hp in (0, 1):
                        for hh in (0, 1):
                            h = 2 * hp + hh
                            lhsT = ats[hp][
                                :, hh * 512 + sbl * 128 : hh * 512 + (sbl + 1) * 128
                            ]
                            for j in (0, 1):
                                nc.tensor.matmul(
                                    ps_op[:, j * 512 : (j + 1) * 512],
                                    lhsT=lhsT,
                                    rhs=woT_sb[
                                        :, h, eg * 1024 + j * 512 : eg * 1024 + (j + 1) * 512
                                    ],
                                    start=(h == 0),
                                    stop=(h == 3),
                                )
                    o_sb = osb_tiles[sbl]
                    ev = (nc.vector.tensor_copy
                          if (ev_dve or eg % 2 == 0) else nc.scalar.copy)
                    ev(o_sb[:, eg * 1024 : (eg + 1) * 1024], ps_op)
                    # per-eg DMAs so output transfers trickle out instead of
                    # piling up after the flush; all on the sync queue (idle in
                    # phase B) to keep triggers off the exp-loaded ACT engine
                    nc.sync.dma_start(
                        out=out_d[
                            (4 * t + sbl) * 128 : (4 * t + sbl + 1) * 128,
                            eg * 1024 : (eg + 1) * 1024,
                        ],
                        in_=o_sb[:, eg * 1024 : (eg + 1) * 1024],
                    )

                pending = []  # oproj closures for the previous chunk
                prev_ats = None
                prev_t = None
                # chunk 1 first (better ACT/PE balance with no oproj filler);
                # chunk 0 last, where chunk 3's oproj fills its short kb loops
                for t in (1, 2, 3, 0):
                    nkb = 4 * (t + 1)
                    if prev_ats is not None:
                        pats = prev_ats
                        pt_ = prev_t
                        pending = [
                            (pt_, sbl, eg, pats)
                            for sbl in range(4)
                            for eg in range(NEG)
                        ]
                        osb_tiles = [
                            pb.tile([128, DIM], dt.float16, name=f"osb_{t}_{sbl}",
                                    tag=f"osb{sbl}", bufs=1)
                            for sbl in range(4)
                        ]
                    at_tiles = []
                    for hp in (0, 1):
                        h0, h1 = 2 * hp, 2 * hp + 1
                        q0 = qt_sb[:, h0, t * 512 : (t + 1) * 512]
                        q1 = qt_sb[:, h1, t * 512 : (t + 1) * 512]
                        ps_o = pbp.tile([128, 1024], dt.float32, tag="pvo", bufs=1)
                        # cp (outer) pool: no address overlap with phase A's
                        # SBUF, so chunk-1 work starts before the pa pool closes
                        dacc = cp.tile([128, 1024], dt.float16, tag="dacc", bufs=2,
                                       name=f"dacc_{t}_{hp}")
                        pt_tiles = [None] * nkb

                        def emit_scores(kb):
                            # diagonal blocks only need q >= k: skip the dead
                            # upper-triangle 128-col strips
                            lo = max(kb - 4 * t, 0) * 128
                            ps_s = pbp.tile(
                                [128, 1024], dt.float32, tag="scores", bufs=2
                            )
                            nc.tensor.matmul(
                                ps_s[:, lo:512],
                                lhsT=kt_sb[:, kb * 128 : (kb + 1) * 128],
                                rhs=q0[:, lo:512], start=True, stop=True,
                            )
                            nc.tensor.matmul(
                                ps_s[:, 512 + lo : 1024],
                                lhsT=kt_sb[:, kb * 128 : (kb + 1) * 128],
                                rhs=q1[:, lo:512], start=True, stop=True,
                            )
                            pt = cp.tile([128, 1024], dt.float16, tag="pt", bufs=4,
                                         name=f"pt_{t}_{hp}_{kb}")
                            if lo == 0:
                                nc.scalar.activation(pt, ps_s, Exp, scale=SCALE, bias=ebias_sb)
                            else:
                                nc.scalar.activation(
                                    pt[:, lo:512], ps_s[:, lo:512],
                                    Exp, scale=SCALE, bias=ebias_sb,
                                )
                                nc.scalar.activation(
                                    pt[:, 512 + lo : 1024], ps_s[:, 512 + lo : 1024],
                                    Exp, scale=SCALE, bias=ebias_sb,
                                )
                            pt_tiles[kb] = pt

                        def emit_mask_dacc(kb):
                            pt = pt_tiles[kb]
                            j = kb - 4 * t
                            lo = max(j, 0) * 128
                            if j >= 0:  # triangular 128-block on the diagonal
                                nc.vector.tensor_mul(
                                    pt[:, lo : lo + 128], pt[:, lo : lo + 128], mask_sb
                                )
                                nc.vector.tensor_mul(
                                    pt[:, 512 + lo : 512 + lo + 128],
                                    pt[:, 512 + lo : 512 + lo + 128], mask_sb,
                                )
                            if kb == 0:
                                nc.vector.tensor_copy(dacc, pt)
                            elif lo == 0:
                                nc.vector.tensor_add(dacc, dacc, pt)
                            else:
                                nc.vector.tensor_add(
                                    dacc[:, lo:512], dacc[:, lo:512], pt[:, lo:512]
                                )
                                nc.vector.tensor_add(
                                    dacc[:, 512 + lo : 1024],
                                    dacc[:, 512 + lo : 1024], pt[:, 512 + lo : 1024],
                                )

                        emit_scores(0)
                        for kb in range(nkb):
                            if kb + 1 < nkb:
                                emit_scores(kb + 1)
                            emit_mask_dacc(kb)
                            pt = pt_tiles[kb]
                            lo = max(kb - 4 * t, 0) * 128
                            # extra oproj groups at the hp boundary to cover
                            # the denominator->at chain of the previous hp
                            for _ in range(3 if kb == 0 else 1):
                                if pending:
                                    oproj_group(*pending.pop(0))
                            for j in (0, 1):
                                nc.tensor.matmul(
                                    ps_o[:, j * 512 + lo : (j + 1) * 512],
                                    lhsT=v_sb[:, kb, :],
                                    rhs=pt[:, j * 512 + lo : (j + 1) * 512],
                                    start=(kb == 0),
                                    stop=(kb == nkb - 1),
                                    skip_group_check=True,
                                )
                        # softmax denominator: every row of ps_d = column sum
                        ps_d = pbp.tile([128, 1024], dt.float32, tag="scores", bufs=2)
                        for j in (0, 1):
                            nc.tensor.matmul(
                                ps_d[:, j * 512 : (j + 1) * 512],
                                lhsT=ones_sb,
                                rhs=dacc[:, j * 512 : (j + 1) * 512],
                                start=True, stop=True,
                            )
                        recip = pb.tile([128, 1024], dt.float32, tag="recip")
                        nc.vector.reciprocal_approx_fast(recip, ps_d)
                        at = pb.tile([128, 1024], dt.float16, tag=f"at{hp}", bufs=2)
                        nc.vector.tensor_mul(at, ps_o, recip)
                        at_tiles.append(at)
                    while pending:
                        oproj_group(*pending.pop(0))
                    prev_ats = at_tiles
                    prev_t = t
                # flush last chunk's oproj
                osb_tiles = [
                    pb.tile([128, DIM], dt.float16, name=f"osb_f_{sbl}",
                            tag=f"osb{sbl}", bufs=1)
                    for sbl in range(4)
                ]
                # rotate psum tags so the flush triple-buffers (pvo and scores
                # are free once the last at-mul is done)
                ftags = ("oproj", "pvo", "scores")
                for i, (sbl, eg) in enumerate(
                    (s_, e_) for s_ in range(4) for e_ in range(NEG)
                ):
                    oproj_group(prev_t, sbl, eg, prev_ats, ptag=ftags[i % 3],
                                ev_dve=True)
    nc.compile()
    return nc


def _prep_inputs(x, cos, sin, wq, wk, wv, wo):
    x = np.asarray(x, dtype=np.float32)
    cos = np.asarray(cos, dtype=np.float32)
    sin = np.asarray(sin, dtype=np.float32)
    wq = np.asarray(wq, dtype=np.float32)
    wk = np.asarray(wk, dtype=np.float32)
    wv = np.asarray(wv, dtype=np.float32)
    wo = np.asarray(wo, dtype=np.float32)

    xt = np.ascontiguousarray(x[0].T).astype(f16)  # [DIM, S]
    # cos/sin transposed, duplicated into both partition halves [128, S];
    # sin sign-folded: rows 0:64 = -sin (pairs with b), rows 64:128 = +sin
    csd = np.tile(cos.T, (2, 1)).astype(f16)
    snd = np.concatenate([-sin.T, sin.T], axis=0).astype(f16)
    # de-interleave perm: head dim pairs (2i, 2i+1) -> rows (i, 64+i)
    perm = np.concatenate([np.arange(0, HD, 2), np.arange(1, HD, 2)])

    # triangular mask for the diagonal 128x128 sub-block: keep k <= q
    r = np.arange(128)[:, None]
    c = np.arange(128)[None, :]
    mask = (r <= c).astype(f16)  # [128, 128]

    in_maps = []
    for i in range(N_CORES):
        wq_i = wq[DQ * i : DQ * (i + 1)]  # [512, DIM]
        wk_i = wk[HD * i : HD * (i + 1)]  # [128, DIM]
        wv_i = wv[HD * i : HD * (i + 1)]
        wq_p = wq_i.reshape(QH_PER_CORE, HD, DIM)[:, perm, :].reshape(DQ, DIM)
        wk_p = wk_i[perm, :]
        w_all = np.concatenate([wq_p, wk_p, wv_i], axis=0)  # [768, DIM]
        # layout [c=128, dtile, cb, n=128]: element (dtile*128+n, cb*128+c)
        wt = (
            w_all.reshape(6, 128, CB, 128)  # [dtile, n, cb, c]
            .transpose(3, 0, 2, 1)  # [c, dtile, cb, n]
            .reshape(128, 6 * CB * 128)
            .astype(f16)
        )
        wot = np.ascontiguousarray(wo[:, DQ * i : DQ * (i + 1)].T).astype(f16)
        in_maps.append(
            {
                "xt": xt,
                "wt": np.ascontiguousarray(wt),
                "wot": wot,
                "csd": np.ascontiguousarray(csd),
                "snd": np.ascontiguousarray(snd),
                "mask": np.ascontiguousarray(mask),
            }
        )
    return in_maps


def _get_runner():
    global _RUNNER
    if _RUNNER is None:
        _RUNNER = _build()
    return _RUNNER


def kernel(x, cos, sin, wq, wk, wv, wo):
    from concourse.bass_utils import run_bass_kernel_spmd

    nc = _get_runner()
    in_maps = _prep_inputs(x, cos, sin, wq, wk, wv, wo)
    res = run_bass_kernel_spmd(nc, in_maps, list(range(N_CORES)))
    out = np.zeros((S, DIM), dtype=np.float32)
    for i in range(N_CORES):
        out += res.results[i]["out"].astype(np.float32)
    return out[None].astype(np.float32)
